# revision 1
# baseline (speedup 1.0000x reference)
"""TRN2 Bass kernel for nn_Attender:
    weights[b, s] = sum_d (state @ W.T + bias)[b, d] * enc[s, b, d]
with enc [S=2048, B=16, D=2048], state [B, D], W [D, D], bias [D], out [B, S].

Sharding (8 NeuronCores): the contraction dim D is split into 8 slices of 256,
one per core. Each core computes alteredT[d_k, b] = (W[d_k, :] @ state.T +
bias[d_k]) — needing only a 256-row slice of W — and the partial score
partial_k[b, s] = sum_{d in d_k} altered[b, d] * enc[s, b, d]. The host sums
the 8 partials (a pure reduction un-shard); no cross-device communication.

The kernel is HBM-stream-bound: ~17.9 MB/core (enc fp16 16.8 MB + W slice
1 MB) at the ~400-425 GB/s/core HWDGE streaming rate ≈ 44 us, plus ~9 us of
fixed runtime/preamble head and a data-gated tail. Measured invariant: the
stream runs gapless and ends at (first-descriptor time + total bytes/rate)
regardless of queue assignment, so the design minimizes the tail instead:

  * Everything inbound rides the sync HWDGE queue in order: constants
    (wp/sp/bk — leading, so the altered-state matmuls start at ~13 us
    deterministically), then 3 big 4-batch tilesets (batches 0-11), then 5
    s-tile pieces covering batches 12-15 together, tapered 1 MB x3,
    0.75 MB, 0.25 MB, so the final transfer — and the compute it gates —
    is small. Secondary
    queues are avoided for inputs: both SWDGE and a second HWDGE ring get
    starved to ~25 GB/s by the saturated sync ring's packet arbitration
    (measured), which can delay wp to ~35 us and stall the MM pipeline.
  * Matmuls are col-group-interleaved: the 4 batches of a PSUM group sit at
    array columns {0,32,64,96} (tile_position) and consecutive MMs cycle
    through them, so 4 MMs stream concurrently through disjoint 32-col
    sub-arrays (~3x PE throughput; measured: 4 MMs start within 10 ns).
  * Dependency-free "warmup" matmuls (into a scratch PSUM bank) are issued
    between the piece blocks so the PE never idles long enough for the HAM
    clock gate to re-throttle it to 1.2 GHz — the final piece's matmuls run
    at the warm 2.4 GHz rate. (Measured cold-vs-warm on the tail: ~0.5 us.)
  * Tail engine budget: piece drains all on DVE, out DMAs on the scalar
    ring in readiness order, except the second-to-last which rides the
    by-then-idle sync engine — so the final [4,128] out's fast scalar
    issue starts ~30 ns after its drain copy (measured) instead of
    queuing behind a prior issue. A variant with more tail scalar ops
    measured 1.5 us slower.

Device layout — partition-major, so each DMA is one contiguous DRAM run
per partition (32 KB packets; measured faster + simpler than chunk-major):
  encA [128, 2*12*S]   batches 0-11:  [p, (tileset, c, b_local, s)]
  encB [128, 2*4*S]    batches 12-15: [p, (piece, c, b_local, s_cols)]
                       pieces = s-ranges (0,512),(512,512),(1024,512),
                       (1536,384),(1920,128) of the last group's S axis
  wp   [128, 16*256]   wp[p, i*256+d] = W[k*256+d, i*128+p]   (lhsT tiles)
  sp   [128, 16*16]    sp[p, i*16+b]  = state[b, i*128+p]
  bk   [128, 2]        bk[p, c]       = bias[k*256 + c*128 + p]  (fp32)

Precision: enc/W/state/altered in fp16, fp32 PSUM accumulate. Measured
error: max|err| = 1.5e-3 * rms(ref) — pure input-rounding, far under the
2e-2 gate.
"""

import os
from contextlib import ExitStack

import numpy as np

import concourse.bacc as bacc
import concourse.tile as tile
import concourse.mybir as mybir
from concourse.bass_utils import run_bass_kernel_spmd

S, B, D = 2048, 16, 2048
NCORES = 8
DK = D // NCORES  # 256 contraction elems per core
NCH = DK // 128  # 2 partition chunks
BG = 4  # batches per psum group
NG = B // BG  # 4 groups
ST = 512  # s-tile (one PSUM bank)
NST = S // ST  # 4 s-tiles
NBA = 12  # batches in region A (big tilesets)
TS_A = 4  # batches per region-A tileset
# Region B pieces: (s_start, s_cols) within the last group's S axis.
# Tapered: the final 0.25 MB pieces shorten the data-gated tail chain
# (matmul N and drain-copy width scale with the last piece's columns).
PIECES = [(0, 512), (512, 512), (1024, 512), (1536, 384), (1920, 128)]
NJUNK = 6  # warmup MMs issued after each non-final piece block

MODE = os.environ.get("BASS_KERNEL_MODE", "fp16x1")

F32 = mybir.dt.float32
F16 = mybir.dt.float16

_CACHE = {}

LAST_RESULTS = None


def _build():
    nc = bacc.Bacc("TRN2", target_bir_lowering=False, debug=False, num_devices=NCORES)

    ENCA = nc.dram_tensor(
        "enca", [128, NCH * NBA * S], F16, kind="ExternalInput"
    ).ap()
    ENCB = nc.dram_tensor(
        "encb", [128, NCH * (B - NBA) * S], F16, kind="ExternalInput"
    ).ap()
    WP = nc.dram_tensor("wp", [128, 16 * DK], F16, kind="ExternalInput").ap()
    SP = nc.dram_tensor("sp", [128, 16 * B], F16, kind="ExternalInput").ap()
    BK = nc.dram_tensor("bk", [128, NCH], F32, kind="ExternalInput").ap()
    OUT = nc.dram_tensor("out", [B, S], F32, kind="ExternalOutput").ap()

    with tile.TileContext(nc) as tc, ExitStack() as ctx:
        cpool = ctx.enter_context(tc.tile_pool(name="const", bufs=1))
        epool = ctx.enter_context(tc.tile_pool(name="enc", bufs=1))
        # One outg buffer per group: recycling (bufs=2) made late groups'
        # PSUM drains wait on earlier groups' output DMAs, which stalls the
        # whole MM pipeline when the scalar DMA ring is starved by the enc
        # stream's packet arbitration.
        opool = ctx.enter_context(tc.tile_pool(name="outp", bufs=4))
        apsum = ctx.enter_context(tc.tile_pool(name="apsum", bufs=1, space="PSUM"))
        mpsum = ctx.enter_context(tc.tile_pool(name="mpsum", bufs=4, space="PSUM"))
        lpsum = ctx.enter_context(tc.tile_pool(name="lpsum", bufs=2, space="PSUM"))

        # Constants lead the sync ring, ahead of the enc stream. On the
        # scalar ring they can be starved to ~25 GB/s by the saturated sync
        # ring's packet arbitration (wp then lands at ~35 us and the whole
        # MM pipeline idles); in-ring they land by ~13 us deterministically,
        # and the measured stream end (first descriptor + total bytes/rate)
        # is unchanged.
        wp_t = cpool.tile([128, 16 * DK], F16, tag="wp")
        nc.sync.dma_start(wp_t[:], WP[:])
        sp_t = cpool.tile([128, 16 * B], F16, tag="sp")
        nc.sync.dma_start(sp_t[:], SP[:])
        bk_t = cpool.tile([128, NCH], F32, tag="bk")
        nc.sync.dma_start(bk_t[:], BK[:])

        # enc stream on the sync HWDGE queue; both d-chunks ride each DMA.
        tsA = []
        tlen = NCH * TS_A * S
        for t in range(NBA // TS_A):
            et = epool.tile([128, tlen], F16, tag=f"enctA{t}", name=f"eA_{t}")
            nc.sync.dma_start(et[:], ENCA[:, t * tlen : (t + 1) * tlen])
            tsA.append(et)
        tsB = []
        boff = 0
        for pi, (s0, scols) in enumerate(PIECES):
            plen = NCH * BG * scols
            et = epool.tile([128, plen], F16, tag=f"encP{pi}", name=f"eB_{pi}")
            nc.sync.dma_start(et[:], ENCB[:, boff : boff + plen])
            boff += plen
            tsB.append(et)

        # alteredT[d, b] = sum_i W[d, i] * state[b, i] + bias[d], d on partitions.
        amats = []  # amats[c] = fp16 lhsT tile [128, B]
        for c in range(NCH):
            aps = apsum.tile([128, B], F32, tag="aps")
            for i in range(16):
                nc.tensor.matmul(
                    aps[:],
                    wp_t[:, i * DK + c * 128 : i * DK + (c + 1) * 128],
                    sp_t[:, i * B : (i + 1) * B],
                    start=(i == 0),
                    stop=(i == 15),
                )
            altf = cpool.tile([128, B], F32, tag=f"altf{c}")
            nc.vector.tensor_scalar_add(altf[:], aps[:], bk_t[:, c : c + 1])
            af = cpool.tile([128, B], F16, tag=f"af{c}")
            nc.vector.tensor_copy(af[:], altf[:])
            amats.append(af)

        # Scratch PSUM bank for dependency-free HAM-warmup matmuls.
        junk = apsum.tile([128, ST], F32, tag="junk")

        out_r = OUT.rearrange("(g bi) s -> g bi s", bi=BG)

        # Groups 0-2 (region A): per group, 4 PSUM banks (one per s-tile);
        # batch bi lands at partition 32*bi of its bank via col tiling; MMs
        # bi-innermost for col-group concurrency; single [4, S] out DMA.
        for g in range(NBA // BG):
            pts = [
                mpsum.tile([128, ST], F32, tag="mm", name=f"pt_{g}_{st}")
                for st in range(NST)
            ]
            for st in range(NST):
                for c in range(NCH):
                    for bi in range(BG):
                        off = (c * TS_A + bi) * S + st * ST
                        nc.tensor.matmul(
                            pts[st][32 * bi : 32 * bi + 1, :],
                            amats[c][:, g * BG + bi : g * BG + bi + 1],
                            tsA[g][:, off : off + ST],
                            start=(c == 0),
                            stop=(c == NCH - 1),
                            tile_position=(0, 32 * bi),
                        )
            outg = opool.tile([128, S], F32, tag="outg", name=f"outg_{g}")
            for st in range(NST):
                dst = outg[:, st * ST : (st + 1) * ST]
                if st % 2 == 0:
                    nc.vector.tensor_copy(dst, pts[st][:])
                else:
                    nc.scalar.copy(dst, pts[st][:])
            src_r = outg[:].rearrange("(bi r) s -> bi r s", bi=BG)[:, 0]
            nc.scalar.dma_start(out_r[g], src_r)

        # Group 3 (region B): compute, drain (DVE), and ship (scalar ring)
        # per piece as each lands. Warmup MMs between pieces keep the PE's
        # HAM clock gate at 8/8 so the final piece computes at 2.4 GHz.
        g = NG - 1
        outg = opool.tile([128, S], F32, tag="outg", name=f"outg_{g}")
        src_r = outg[:].rearrange("(bi r) s -> bi r s", bi=BG)[:, 0]
        for pi, (s0, scols) in enumerate(PIECES):
            pt = lpsum.tile([128, ST], F32, tag="late", name=f"pt_{g}_{pi}")
            for c in range(NCH):
                for bi in range(BG):
                    off = (c * BG + bi) * scols
                    nc.tensor.matmul(
                        pt[32 * bi : 32 * bi + 1, :scols],
                        amats[c][:, g * BG + bi : g * BG + bi + 1],
                        tsB[pi][:, off : off + scols],
                        start=(c == 0),
                        stop=(c == NCH - 1),
                        tile_position=(0, 32 * bi),
                    )
            # The final piece drains on ACT (scalar), not DVE: the DVE is
            # still busy with the previous piece's copy at that point
            # (measured ~0.33 us queue wait), while ACT's last work was two
            # pieces earlier — and its out DMA issues on the same engine
            # right behind the copy with no cross-engine sem hop.
            if pi == len(PIECES) - 1:
                nc.scalar.copy(outg[:, s0 : s0 + scols], pt[:, :scols])
            else:
                nc.vector.tensor_copy(outg[:, s0 : s0 + scols], pt[:, :scols])
            # The second-to-last out rides the (by then idle) sync engine so
            # the scalar NX is free when the final copy lands: the final
            # out's fast scalar issue (487 ns vs sync's 777 ns) then starts
            # right after the copy instead of queuing behind a prior issue.
            eng = nc.sync if pi == len(PIECES) - 2 else nc.scalar
            eng.dma_start(
                out_r[g][:, s0 : s0 + scols], src_r[:, s0 : s0 + scols]
            )
            if pi < 2:
                # Fill the wait for the next piece with dependency-free MMs
                # (all inputs resident since tileset A0) so HAM stays warm.
                # None after the later pieces: their real MM blocks arrive
                # close enough to bridge the idle windows, and a queued
                # warmup MM would delay the final data-gated matmuls.
                for _ in range(NJUNK):
                    nc.tensor.matmul(
                        junk[0:1, :],
                        amats[0][:, 0:1],
                        tsA[0][:, 0:ST],
                        start=True,
                        stop=True,
                        tile_position=(0, 0),
                    )

    nc.compile()
    return nc


def _prep_inputs(encoder_outputs, state, W, b):
    """Build the 8 per-core input maps (heavy layout work on host)."""
    in_maps = []
    # [S, B, D] -> [B, D, S] once
    encT = np.ascontiguousarray(encoder_outputs.transpose(1, 2, 0))
    spk = np.ascontiguousarray(
        state.T.reshape(16, 128, B).transpose(1, 0, 2).reshape(128, 16 * B)
    ).astype(np.float16)
    for k in range(NCORES):
        d0 = k * DK
        e = encT[:, d0 : d0 + DK, :]  # [B, DK, S]
        # -> [c, p, B, S] fp16
        e = (
            np.ascontiguousarray(e.reshape(B, NCH, 128, S).transpose(1, 2, 0, 3))
            .astype(np.float16)
        )
        # region A partition-major: [p, (tileset, c, b_local, s)] so each
        # partition's tileset data is one contiguous DRAM run.
        enc_a = np.ascontiguousarray(
            e[:, :, :NBA, :]
            .reshape(NCH, 128, NBA // TS_A, TS_A, S)
            .transpose(1, 2, 0, 3, 4)
            .reshape(128, NCH * NBA * S)
        )
        # batches 12-15 partition-major: [p, (piece, c, b_local, s_cols)].
        eb = e[:, :, NBA:, :]  # [c, p, 4, S]
        parts = [
            eb[:, :, :, s0 : s0 + scols]
            .transpose(1, 0, 2, 3)
            .reshape(128, NCH * BG * scols)
            for (s0, scols) in PIECES
        ]
        enc_b = np.ascontiguousarray(np.concatenate(parts, axis=1))
        wp = np.ascontiguousarray(
            W[d0 : d0 + DK, :].T.reshape(16, 128, DK).transpose(1, 0, 2).reshape(128, 16 * DK)
        ).astype(np.float16)
        bk = np.ascontiguousarray(b[d0 : d0 + DK].reshape(NCH, 128).T)
        in_maps.append(
            {"enca": enc_a, "encb": enc_b, "wp": wp, "sp": spk, "bk": bk}
        )
    return in_maps


def kernel(encoder_outputs, state, W, b):
    global LAST_RESULTS
    if "k" not in _CACHE:
        _CACHE["k"] = _build()
    nc = _CACHE["k"]
    in_maps = _prep_inputs(
        np.asarray(encoder_outputs, dtype=np.float32),
        np.asarray(state, dtype=np.float32),
        np.asarray(W, dtype=np.float32),
        np.asarray(b, dtype=np.float32),
    )
    res = run_bass_kernel_spmd(nc, in_maps, core_ids=list(range(NCORES)))
    LAST_RESULTS = res
    acc = np.zeros((B, S), dtype=np.float64)
    for k in range(NCORES):
        acc += res.results[k]["out"].astype(np.float64)
    return acc.astype(np.float32)

